# revision 22
# baseline (speedup 1.0000x reference)
"""Causal self-attention (RoPE + QK-RMSNorm, GQA 16q/8kv) Trainium2 Bass kernel.

Sharding: 8 cores = 2 batch x 4 tensor-parallel. Core c handles batch b=c//4 and
q-heads [4*tp, 4*tp+4), kv-heads [2*tp, 2*tp+2) where tp=c%4. Each core returns a
partial (T, C) output = O_heads @ wo[rows of its heads]; host sums the 4 partials
per batch (the "all-reduce after c_proj").

v3 schedule: chunk-projection (C), attention-span (S) and output-projection (P)
phases are interleaved C0 S0 C1 P0 S1 C2 P1 S2 C3 P2 S3 P3 so the PE stream
never drains. Scalar runs only Sqrt/Exp/Copy (few activation-table loads);
reciprocals run on the DVE; elementwise casts/adds off the critical path run on
the otherwise-idle GpSimd. Latency tails (RMS apply broadcasts, last-head
softmax normalization) are deferred into the next phase's independent PE stream.
Diagonal attention blocks are restricted to their valid causal q-range.
"""
import sys
import math

sys.path.insert(0, "/opt/trn_rl_repo")

import numpy as np
import ml_dtypes
import concourse.bacc as bacc
import concourse.mybir as mybir
import concourse.tile as tile
from concourse.bass_utils import run_bass_kernel_spmd

P = 128
T = 2048
C = 2048
KO = C // P          # 16 contraction tiles
D = 128              # head dim
NQ = 4               # q heads per core
NK = 2               # kv heads per core
NF = NQ + NK         # 6 rope/rms feature blocks (4 q + 2 k)
FQ = NQ * D          # 512
FK = NK * D          # 256
TCH = 512            # chunk / span size
NCHUNK = T // TCH    # 4
SPAN = 512
KB = T // P          # 16 key blocks
SCALE = 1.0 / math.sqrt(D)
DEPTH = 3            # score-ahead software pipeline depth in attention

f32 = mybir.dt.float32
bf16 = mybir.dt.bfloat16

AF = mybir.ActivationFunctionType


def build():
    nc = bacc.Bacc("TRN2", target_bir_lowering=False)
    xT = nc.dram_tensor("xT", (C, T), bf16, kind="ExternalInput")
    wq = nc.dram_tensor("wq", (C, FQ), bf16, kind="ExternalInput")
    wk = nc.dram_tensor("wk", (C, FK), bf16, kind="ExternalInput")
    wv = nc.dram_tensor("wv", (C, FK), bf16, kind="ExternalInput")
    wo = nc.dram_tensor("wo", (FQ, C), bf16, kind="ExternalInput")
    cc = nc.dram_tensor("cc", (P, T), bf16, kind="ExternalInput")    # [cos; cos]
    ss = nc.dram_tensor("ss", (P, T), bf16, kind="ExternalInput")    # [sin; -sin]
    mask = nc.dram_tensor("mask", (P, P), bf16, kind="ExternalInput")  # [k, qq] = qq>=k
    y = nc.dram_tensor("y", (T, C), bf16, kind="ExternalOutput")

    xT_r = xT.rearrange("(ko p) t -> p ko t", p=P)
    wq_r = wq.rearrange("(ko p) f -> p ko f", p=P)
    wk_r = wk.rearrange("(ko p) f -> p ko f", p=P)
    wv_r = wv.rearrange("(ko p) f -> p ko f", p=P)
    wo_r = wo.rearrange("(ko p) n -> p ko n", p=P)

    with tile.TileContext(nc) as tc:
        with (
            tc.tile_pool(name="persist", bufs=1) as persist,
            tc.tile_pool(name="otp", bufs=2) as otp,
            tc.tile_pool(name="xp", bufs=2) as xp,
            tc.tile_pool(name="tpf", bufs=2) as tpf,
            tc.tile_pool(name="tps", bufs=2) as tps,
            tc.tile_pool(name="sqp", bufs=6) as sqp,
            tc.tile_pool(name="rstdp", bufs=6) as rstdp,
            tc.tile_pool(name="tpt", bufs=6) as tpt,
            tc.tile_pool(name="tpy", bufs=10) as tpy,
            tc.tile_pool(name="ps_mm", bufs=4, space="PSUM") as ps_mm,
            tc.tile_pool(name="ps_ot", bufs=2, space="PSUM") as ps_ot,
            tc.tile_pool(name="ps_sum", bufs=2, space="PSUM") as ps_sum,
        ):
            qk_rt = persist.tile([P, NF, T], bf16, tag="qk_rt")   # roped+normed qT/kT
            v_sb = persist.tile([P, KB, FK], bf16, tag="v_sb")    # V natural [t-part, kb, feat]
            cc_sb = persist.tile([P, T], bf16, tag="cc_sb")
            ss_sb = persist.tile([P, T], bf16, tag="ss_sb")
            mask_sb = persist.tile([P, P], bf16, tag="mask_sb")
            ones_col = persist.tile([P, 1], bf16, tag="ones_col")    # sums lhsT
            ones_row = persist.tile([1, P], bf16, tag="ones_row")    # bcast lhsT
            ones_f32 = persist.tile([P, 1], f32, tag="ones_f32")
            ones_row_f32 = persist.tile([1, P], f32, tag="ones_row_f32")
            wq_sb = persist.tile([P, KO, FQ], bf16, tag="wq_sb")
            wk_sb = persist.tile([P, KO, FK], bf16, tag="wk_sb")
            wv_sb = persist.tile([P, KO, FK], bf16, tag="wv_sb")
            wo_sb = persist.tile([P, NQ, C], bf16, tag="wo_sb")

            # split weight DMAs so the first matmuls wait only on their slice
            for fb in range(NQ):
                nc.sync.dma_start(wq_sb[:, :, fb * D : (fb + 1) * D],
                                  wq_r[:, :, fb * D : (fb + 1) * D])
            for fb in range(NK):
                nc.sync.dma_start(wk_sb[:, :, fb * D : (fb + 1) * D],
                                  wk_r[:, :, fb * D : (fb + 1) * D])
            nc.sync.dma_start(wv_sb[:], wv_r)
            nc.sync.dma_start(wo_sb[:], wo_r)
            nc.sync.dma_start(cc_sb[:], cc[:, :])
            nc.sync.dma_start(ss_sb[:], ss[:, :])
            nc.sync.dma_start(mask_sb[:], mask[:, :])
            nc.vector.memset(ones_f32[:], 1.0)
            nc.vector.memset(ones_row_f32[:], 1.0)
            nc.vector.tensor_copy(ones_col[:], ones_f32[:])
            nc.vector.tensor_copy(ones_row[:], ones_row_f32[:])

            def prefetch_x(c):
                t0 = c * TCH
                xt = xp.tile([P, KO, TCH], bf16, tag="xt")
                for ko in range(KO):
                    nc.sync.dma_start(xt[:, ko, :], xT_r[:, ko, t0 : t0 + TCH])
                return xt

            def emit_chunk(c, xt, norm_filler=None):
                """Project chunk c -> roped/normalized qT/kT + natural V.
                Returns thunks: deferred RMS-applies for q heads 1..3 (must run
                before span c's head h reads qk_rt[h])."""
                t0 = c * TCH
                segs = [None] * NF

                def emit_fb(fb):
                    if fb < NQ:
                        w_ap = wq_sb[:, :, fb * D : (fb + 1) * D]
                    else:
                        w_ap = wk_sb[:, :, (fb - NQ) * D : (fb - NQ + 1) * D]
                    pqk = ps_mm.tile([P, TCH], f32, tag="ps_mm")
                    for ko in range(KO):
                        nc.tensor.matmul(
                            pqk[:], w_ap[:, ko], xt[:, ko, :],
                            start=(ko == 0), stop=(ko == KO - 1),
                        )
                    # rope: raw copy on Scalar (table-safe), half-swap via DMA
                    raw = tpf.tile([P, TCH], f32, tag="raw")
                    nc.scalar.activation(raw[:], pqk[:], AF.Copy)
                    swp = tpf.tile([P, TCH], f32, tag="swp")
                    nc.sync.dma_start(swp[0:64, :], raw[64:128, :])
                    nc.sync.dma_start(swp[64:128, :], raw[0:64, :])
                    tmpa = tpf.tile([P, TCH], f32, tag="tmpa")
                    tmpb = tpf.tile([P, TCH], f32, tag="tmpb")
                    seg = qk_rt[:, fb, t0 : t0 + TCH]
                    nc.vector.tensor_mul(tmpa[:], pqk[:], cc_sb[:, t0 : t0 + TCH])
                    nc.vector.tensor_mul(tmpb[:], swp[:], ss_sb[:, t0 : t0 + TCH])
                    nc.gpsimd.tensor_add(seg, tmpa[:], tmpb[:])
                    sq = sqp.tile([P, TCH], bf16, tag="sq")
                    nc.vector.tensor_mul(sq[:], seg, seg)
                    segs[fb] = (seg, sq)

                # K features first so their rstd chains finish earliest
                for fb in (4, 5, 0, 1, 2, 3):
                    emit_fb(fb)
                    if fb == 4 and norm_filler is not None:
                        norm_filler()

                rstds = {}

                def emit_stat(fb):
                    pms = ps_sum.tile([1, TCH], f32, tag="ps_sum")
                    nc.tensor.matmul(pms[:], ones_col[:], segs[fb][1][:], start=True, stop=True)
                    # rstd = 1/sqrt(ms) = sqrt(D / pms); eps is negligible vs ms
                    inv = tps.tile([1, TCH], f32, tag="inv")
                    nc.vector.reciprocal_approx_fast(inv[:], pms[:])
                    rstd = rstdp.tile([1, TCH], bf16, tag="rstd")
                    nc.scalar.activation(rstd[:], inv[:], AF.Sqrt, scale=float(D))
                    rstds[fb] = rstd

                def emit_apply(fb):
                    pb = ps_mm.tile([P, TCH], f32, tag="ps_mm")
                    nc.tensor.matmul(pb[:], ones_row[:], rstds[fb][:], start=True, stop=True)
                    seg = segs[fb][0]
                    nc.vector.tensor_mul(seg, seg, pb[:])

                def emit_v(tb):
                    pv = ps_mm.tile([P, TCH], f32, tag="ps_mm")
                    for ko in range(KO):
                        nc.tensor.matmul(
                            pv[:, :FK],
                            xt[:, ko, tb * P : (tb + 1) * P],
                            wv_sb[:, ko, :],
                            start=(ko == 0), stop=(ko == KO - 1),
                        )
                    nc.vector.tensor_copy(
                        v_sb[:, c * (TCH // P) + tb, :], pv[:, :FK]
                    )

                # stats for K heads first (span c's scores need K normalized),
                # V matmuls as PE filler over the rstd latency chains
                emit_v(0)
                emit_stat(4)
                emit_v(1)
                emit_stat(5)
                emit_v(2)
                emit_stat(0)
                emit_apply(4)
                emit_v(3)
                emit_stat(1)
                emit_apply(5)
                emit_stat(2)
                emit_apply(0)
                emit_stat(3)
                deferred = [lambda fb=fb: emit_apply(fb) for fb in (1, 2, 3)]
                return deferred

            def emit_span(s, fillers):
                """Attention for q-span s. `fillers` are independent PE thunks
                sprinkled into the score stream (popped front-first). Returns
                the deferred normalization thunk of the last head."""
                q0 = s * SPAN
                nkb = 4 * s + 4
                ot_t = otp.tile([P, NQ, SPAN], bf16, tag="ot_t")
                pending = []

                def emit_norm(h, ot_ps, rec_r):
                    bc = ps_mm.tile([P, SPAN], f32, tag="ps_mm")
                    nc.tensor.matmul(bc[:], ones_row[:], rec_r[:], start=True, stop=True)
                    bc_sb = tps.tile([P, SPAN], f32, tag="bc_sb")
                    nc.vector.tensor_copy(bc_sb[:], bc[:])
                    nc.vector.tensor_mul(ot_t[:, h, :], ot_ps[:], bc_sb[:])

                for h in range(NQ):
                    j = h // 2
                    ot_ps = ps_ot.tile([P, SPAN], f32, tag="ot_ps")
                    sum_ps = ps_sum.tile([1, SPAN], f32, tag="ps_sum")
                    queue = []

                    def flush_one():
                        kb, off, vq, pt = queue.pop(0)
                        nc.tensor.matmul(
                            ot_ps[:, off:],
                            v_sb[:, kb, j * D : (j + 1) * D],
                            pt[:, :vq],
                            start=(kb == 0), stop=(kb == nkb - 1),
                            skip_group_check=True,
                        )
                        nc.tensor.matmul(
                            sum_ps[:, off:],
                            ones_col[:],
                            pt[:, :vq],
                            start=(kb == 0), stop=(kb == nkb - 1),
                            skip_group_check=True,
                        )

                    for kb in range(nkb):
                        r = kb - 4 * s           # >=0: diagonal block
                        off = P * r if r > 0 else 0
                        vq = SPAN - off
                        st = ps_mm.tile([P, SPAN], f32, tag="ps_mm")
                        nc.tensor.matmul(
                            st[:, :vq],
                            qk_rt[:, NQ + j, kb * P : (kb + 1) * P],
                            qk_rt[:, h, q0 + off : q0 + SPAN],
                            start=True, stop=True,
                        )
                        pt = tpt.tile([P, SPAN], bf16, tag="pt")
                        nc.scalar.activation(pt[:, :vq], st[:, :vq], AF.Exp, scale=SCALE)
                        if r >= 0:
                            nc.vector.tensor_mul(pt[:, :P], pt[:, :P], mask_sb[:])
                        queue.append((kb, off, vq, pt))
                        if fillers:
                            fillers.pop(0)()
                        if len(queue) > DEPTH:
                            flush_one()
                        if kb == DEPTH - 1 and pending:
                            emit_norm(*pending.pop())
                    while queue:
                        flush_one()
                    # DVE part of softmax normalization; the PE broadcast is
                    # deferred into the next head's (or phase's) PE stream
                    rec = tps.tile([1, SPAN], f32, tag="rec")
                    nc.vector.reciprocal_approx_fast(rec[:], sum_ps[:])
                    rec_r = tps.tile([1, SPAN], bf16, tag="rec_r")
                    nc.vector.tensor_copy(rec_r[:], rec[:])
                    pending.append((h, ot_ps, rec_r))
                last = pending.pop()
                return ot_t, (lambda: emit_norm(*last))

            def proj_thunks(c, ot_t):
                """Output projection for span c as independent PE thunks."""
                def one(tb, nch):
                    yps = ps_mm.tile([P, 512], f32, tag="ps_mm")
                    for h in range(NQ):
                        nc.tensor.matmul(
                            yps[:],
                            ot_t[:, h, tb * P : (tb + 1) * P],
                            wo_sb[:, h, nch * 512 : (nch + 1) * 512],
                            start=(h == 0), stop=(h == NQ - 1),
                        )
                    ysb = tpy.tile([P, 512], bf16, tag="ysb")
                    nc.vector.tensor_copy(ysb[:], yps[:])
                    nc.sync.dma_start(
                        y[(4 * c + tb) * P : (4 * c + tb + 1) * P,
                          nch * 512 : (nch + 1) * 512],
                        ysb[:],
                    )
                return [lambda tb=tb, nch=nch: one(tb, nch)
                        for tb in range(4) for nch in range(C // 512)]

            # C0 S0 C1 S1(+P0) C2 S2(+P1) C3 S3(+P2) P3: the output
            # projections ride as fillers inside the next span's score
            # stream, soaking up exp-latency bubbles.
            x0 = prefetch_x(0)
            d0 = emit_chunk(0, x0)
            x1 = prefetch_x(1)
            ot0, n0 = emit_span(0, d0)
            d1 = emit_chunk(1, x1, norm_filler=n0)
            x2 = prefetch_x(2)
            ot1, n1 = emit_span(1, d1 + proj_thunks(0, ot0))
            d2 = emit_chunk(2, x2, norm_filler=n1)
            x3 = prefetch_x(3)
            ot2, n2 = emit_span(2, d2 + proj_thunks(1, ot1))
            d3 = emit_chunk(3, x3, norm_filler=n2)
            ot3, n3 = emit_span(3, d3 + proj_thunks(2, ot2))
            n3()
            for t in proj_thunks(3, ot3):
                t()
    nc.compile()
    return nc


_NC_CACHE = None


def _get_nc():
    global _NC_CACHE
    if _NC_CACHE is None:
        _NC_CACHE = build()
    return _NC_CACHE


def _host_inputs(x, cos, sin, wq, wk, wv, wo):
    """Build the 8 per-core input maps."""
    bft = ml_dtypes.bfloat16
    cosT = np.ascontiguousarray(cos[0, :, 0, :].T).astype(np.float32)  # (64, T)
    sinT = np.ascontiguousarray(sin[0, :, 0, :].T).astype(np.float32)
    cc = np.concatenate([cosT, cosT], axis=0).astype(bft)  # (128, T)
    ss = np.concatenate([sinT, -sinT], axis=0).astype(bft)
    # mask[k, qq] = 1 if qq >= k (within the 128-wide diagonal sub-block)
    qq = np.arange(P)[None, :]
    kk = np.arange(P)[:, None]
    mask = (qq >= kk).astype(bft)  # (128, 128)

    xTs = [np.ascontiguousarray(x[b].T).astype(bft) for b in range(2)]
    wq16 = wq.astype(bft)
    wk16 = wk.astype(bft)
    wv16 = wv.astype(bft)
    wo16 = wo.astype(bft)
    in_maps = []
    for c in range(8):
        b, tp = divmod(c, 4)
        in_maps.append(
            {
                "xT": xTs[b],
                "wq": np.ascontiguousarray(wq16[:, tp * FQ : (tp + 1) * FQ]),
                "wk": np.ascontiguousarray(wk16[:, tp * FK : (tp + 1) * FK]),
                "wv": np.ascontiguousarray(wv16[:, tp * FK : (tp + 1) * FK]),
                "wo": np.ascontiguousarray(wo16[tp * FQ : (tp + 1) * FQ, :]),
                "cc": cc,
                "ss": ss,
                "mask": mask,
            }
        )
    return in_maps


def kernel(x, cos, sin, wq, wk, wv, wo, trace=False):
    x = np.asarray(x, dtype=np.float32)
    cos = np.asarray(cos, dtype=np.float32)
    sin = np.asarray(sin, dtype=np.float32)
    wq = np.asarray(wq, dtype=np.float32)
    wk = np.asarray(wk, dtype=np.float32)
    wv = np.asarray(wv, dtype=np.float32)
    wo = np.asarray(wo, dtype=np.float32)

    nc = _get_nc()
    in_maps = _host_inputs(x, cos, sin, wq, wk, wv, wo)
    res = run_bass_kernel_spmd(nc, in_maps, core_ids=list(range(8)), trace=trace)
    out = np.zeros((2, T, C), dtype=np.float32)
    for c in range(8):
        b = c // 4
        out[b] += res.results[c]["y"].astype(np.float32)
    if trace:
        return out, res
    return out
